# revision 2
# baseline (speedup 1.0000x reference)
import sys
sys.path.insert(0, "/opt/trn_rl_repo")
import numpy as np
import ml_dtypes
from contextlib import ExitStack

import concourse.bass as bass
import concourse.bacc as bacc
import concourse.tile as tile
from concourse import mybir
from concourse.bass_utils import run_bass_kernel_spmd

F32 = mybir.dt.float32
F32R = mybir.dt.float32r
BF16 = mybir.dt.bfloat16
FP8 = mybir.dt.float8e4
AF = mybir.ActivationFunctionType
OP = mybir.AluOpType
DR = mybir.MatmulPerfMode.DoubleRow

B, NQ, NK, DV, H, D = 4, 1024, 1024, 1024, 16, 64
QS = 512
EPS = 1e-5
SCALE = 1.0 / 32.0
OUT_SHAPE = (QS, DV)

_CACHE = {}
PHASES = []
DR_PROJ = True   # fp8 DoubleRow for Q/K/V projections
FP8_CTX = True   # fp8 at/vp + DR ctx
OFFLOAD = False  # exp offload to DVE/Pool via quadratic
DEBUG_DUMP = False


def expected_shard(full, c):
    b, q0 = c // 2, (c % 2) * QS
    return np.asarray(full)[b, q0:q0 + QS, :]


def _build(zero_bias, dr_proj=None, fp8_ctx=None):
    nc = bacc.Bacc("TRN2", target_bir_lowering=False)
    dr_proj = DR_PROJ if dr_proj is None else dr_proj
    fp8_ctx = FP8_CTX if fp8_ctx is None else fp8_ctx
    PN = FP8 if dr_proj else BF16
    PA = FP8 if fp8_ctx else BF16
    PHASES.clear()

    def ph(label):
        PHASES.append((label, nc.next_id()))

    qt_d = nc.dram_tensor("qt", [DV, QS], BF16, kind="ExternalInput")
    kt_d = nc.dram_tensor("kt", [DV, NK], BF16, kind="ExternalInput")
    qtok_d = nc.dram_tensor("qtok", [QS, DV], F32, kind="ExternalInput")
    # fp8 weights packed for DoubleRow: [512, 2048], row block j holds
    # [128, 2, 1024] = (partition, subtile-pair slot, dv)
    wq_d = nc.dram_tensor("wq", [512, 2048], PN, kind="ExternalInput")
    wk_d = nc.dram_tensor("wk", [512, 2048], PN, kind="ExternalInput")
    wv_d = nc.dram_tensor("wv", [512, 2048], PN, kind="ExternalInput")
    wo_d = nc.dram_tensor("wo", [DV, DV], BF16, kind="ExternalInput")
    idn_d = nc.dram_tensor("idn", [128, 128], BF16, kind="ExternalInput")
    out_d = nc.dram_tensor("out", [QS, DV], F32, kind="ExternalOutput")
    if DEBUG_DUMP:
        dbg_qp = nc.dram_tensor("dbg_qp", [128, QS], BF16, kind="ExternalOutput")
        dbg_kp = nc.dram_tensor("dbg_kp", [128, NK], BF16, kind="ExternalOutput")
        dbg_at = nc.dram_tensor("dbg_at", [128, 1024], PA, kind="ExternalOutput")
        dbg_vp = nc.dram_tensor("dbg_vp", [128, 2080], PA, kind="ExternalOutput")
        dbg_ot = nc.dram_tensor("dbg_ot", [128, QS], F32, kind="ExternalOutput")
        dbg_on = nc.dram_tensor("dbg_on", [128, NK], BF16, kind="ExternalOutput")
        dbg_qn = nc.dram_tensor("dbg_qn", [128, 1024], PN, kind="ExternalOutput")

    with tile.TileContext(nc) as tc, ExitStack() as ctx:
        ctx.enter_context(nc.allow_low_precision(reason="bf16/fp8 by design"))
        P = ctx.enter_context
        pool = P(tc.tile_pool(name="main", bufs=1))
        pw = P(tc.tile_pool(name="w", bufs=12))        # q/k/v weights [128,2048]
        pqt = P(tc.tile_pool(name="pqt", bufs=8))      # qt bf16 [128,512]
        pot = P(tc.tile_pool(name="pot", bufs=8))      # otok f32 [128,512]
        pbig = P(tc.tile_pool(name="big", bufs=8))     # kt bf16 [128,1024]
        pqk = P(tc.tile_pool(name="qtok", bufs=4))     # qtok f32 [128,1024]
        pqn = P(tc.tile_pool(name="qn", bufs=4))       # qn8 [128,1024] fp8
        pkn = P(tc.tile_pool(name="kn", bufs=4))       # kn8 [128,2048] fp8
        pon = P(tc.tile_pool(name="on", bufs=4))       # on bf16 [128,1024]
        pqp = P(tc.tile_pool(name="qp", bufs=8))       # qp bf16 [128,512]
        pkp = P(tc.tile_pool(name="kp", bufs=8))       # kp bf16 [128,1024]
        pvp = P(tc.tile_pool(name="vp", bufs=4))       # vp8 [128,2080] fp8
        pat = P(tc.tile_pool(name="at", bufs=24 if fp8_ctx else 4))  # at tiles
        pxs = P(tc.tile_pool(name="xs", bufs=1))       # exp-offload scratch f32
        pbc = P(tc.tile_pool(name="bc", bufs=4 if (dr_proj and fp8_ctx) else 2))
        psq = P(tc.tile_pool(name="sq", bufs=3 if (dr_proj and fp8_ctx) else 2))
        pcol = P(tc.tile_pool(name="col", bufs=28))    # [128,1] f32
        pout = P(tc.tile_pool(name="pout", bufs=2))    # out staging f32 [128,512]
        pwo = P(tc.tile_pool(name="pwo", bufs=8))      # wo bf16 [128,1024]
        pp = P(tc.tile_pool(name="pp", bufs=2, space="PSUM"))
        pss = P(tc.tile_pool(name="pss", bufs=2, space="PSUM"))
        pcc = P(tc.tile_pool(name="pcc", bufs=2, space="PSUM"))

        # ---- tiny constants (no DMA where avoidable) ----
        if not zero_bias:
            ones16 = pool.tile([1, 128], BF16, tag="ones16")
            nc.vector.memset(ones16[:], 1.0)
        if OFFLOAD:
            ones16x = pool.tile([1, 128], BF16, tag="ones16x")
            nc.vector.memset(ones16x[:], 1.0)
            halfcol8 = pool.tile([128, 2], FP8, tag="halfcol8")
            nc.vector.memset(halfcol8[:], 0.5)
        onescol = pool.tile([128, 1], BF16, tag="onescol")
        nc.vector.memset(onescol[:], 1.0 / DV)
        onescolr = pool.tile([128, 1], F32R, tag="onescolr")
        nc.vector.memset(onescolr[:].bitcast(F32), 1.0 / DV)
        epst = pool.tile([1, 1], F32, tag="epst")
        nc.vector.memset(epst[:], EPS)
        epsc = pool.tile([128, 1], F32, tag="epsc")
        nc.vector.memset(epsc[:], EPS)
        mrow_q = pool.tile([1, QS], BF16, tag="mrow_q")
        irow_q = pool.tile([1, QS], BF16, tag="irow_q")
        mrow_k = pool.tile([1, NK], BF16, tag="mrow_k")
        irow_k = pool.tile([1, NK], BF16, tag="irow_k")
        m32r = pool.tile([1, 512], F32, tag="m32r")
        varr = pool.tile([1, 512], F32, tag="varr")
        sr = pool.tile([1, 512], F32, tag="sr")

        # ---- input DMAs in consumer order ----
        kt = [pbig.tile([128, NK], BF16, tag="kt", name=f"kt{f}") for f in range(8)]
        for f in range(8):
            nc.sync.dma_start(kt[f][:], kt_d[f * 128:(f + 1) * 128, :])
        qt = [pqt.tile([128, QS], BF16, tag="pq", name=f"qt{f}") for f in range(8)]
        for f in range(8):
            nc.sync.dma_start(qt[f][:], qt_d[f * 128:(f + 1) * 128, :])
        wq = [pw.tile([128, 2048], PN, tag="w", name=f"wq{j}") for j in range(4)]
        for j in range(4):
            nc.sync.dma_start(wq[j][:], wq_d[j * 128:(j + 1) * 128, :])
        wk = [pw.tile([128, 2048], PN, tag="w", name=f"wk{j}") for j in range(4)]
        for j in range(4):
            nc.sync.dma_start(wk[j][:], wk_d[j * 128:(j + 1) * 128, :])
        wv = [pw.tile([128, 2048], PN, tag="w", name=f"wv{j}") for j in range(4)]
        for j in range(4):
            nc.sync.dma_start(wv[j][:], wv_d[j * 128:(j + 1) * 128, :])

        def w3(t):
            return t[:].rearrange("p (s n) -> p s n", s=2)

        def stats_sums(xtiles, cs, n_sq, sq_eng=None, f_range=None, psum=None,
                       use_ss=False):
            if psum is None:
                if use_ss:
                    big = pss.tile([128, 1024], F32, tag="ss")
                    s0t, s1t = big[:, 0:512], big[:, 512:1024]
                else:
                    s0t = pp.tile([128, 512], F32, tag="pp")
                    s1t = pp.tile([128, 512], F32, tag="pp")
                for f in range(8):
                    nc.tensor.matmul(s0t[0:1, :], onescol[:, 0:1], xtiles[f][:, cs],
                                     start=(f == 0), stop=(f == 7))
            else:
                s0t, s1t = psum
            for f in (f_range or range(8)):
                sq = psq.tile([128, 512], F32R, tag="sq", name=f"sq{n_sq}_{f}")
                if sq_eng == "act":
                    nc.scalar.activation(sq[:], xtiles[f][:, cs], AF.Square)
                else:
                    nc.gpsimd.tensor_mul(sq[:], xtiles[f][:, cs], xtiles[f][:, cs])
                nc.tensor.matmul(s1t[0:1, :], onescolr[:, 0:1], sq[:],
                                 start=(f == 0), stop=(f == 7))
            return s0t, s1t

        def stats_rows(s0t, s1t, mrow, irow):
            nc.scalar.activation(mrow[:], s0t[0:1, :], AF.Copy)
            m2 = m32r[0:1, 0:512]
            nc.scalar.activation(m2, s0t[0:1, :], AF.Square)
            var = varr[0:1, 0:512]
            nc.vector.tensor_sub(var, s1t[0:1, :], m2)
            srow = sr[0:1, 0:512]
            nc.scalar.activation(srow, var, AF.Sqrt, bias=epst[:])
            nc.vector.reciprocal(irow[:], srow)

        def bcast(mrow, irow, nm):
            bmt = pbc.tile([128, 512], BF16, tag="bc", name=f"bm{nm}")
            nc.gpsimd.partition_broadcast(bmt[:], mrow[:])
            bit = pbc.tile([128, 512], BF16, tag="bc", name=f"bi{nm}")
            nc.gpsimd.partition_broadcast(bit[:], irow[:])
            return bmt, bit

        # ---- stats + prenorm, finely interleaved for the ramp ----
        ph('statsq')
        kn8 = [pkn.tile([128, 2048], PN, tag="kn", name=f"kn8_{j}") for j in range(4)]
        qn8 = [pqn.tile([128, 1024], PN, tag="qn", name=f"qn8_{j}") for j in range(4)]

        def pren_q():
            for f in range(8):
                j, s = f // 2, f % 2
                dst = qn8[j][:, s * 512:(s + 1) * 512]
                eng = nc.gpsimd if f % 2 == 0 else nc.vector
                eng.tensor_sub(dst, qt[f][:], bmq[:])
                eng.tensor_mul(dst, dst, biq[:])

        def pren_k(c, bmk, bik):
            cs = slice(c * 512, (c + 1) * 512)
            for f in range(8):
                j, s = f // 2, f % 2
                dst = kn8[j][:, s * 1024 + c * 512: s * 1024 + (c + 1) * 512]
                eng = nc.gpsimd if f % 2 == 0 else nc.vector
                eng.tensor_sub(dst, kt[f][:, cs], bmk[:])
                eng.tensor_mul(dst, dst, bik[:])

        ph('statsk')
        s0k0, s1k0 = stats_sums(kt, slice(0, 512), 1)
        s0k1, s1k1 = stats_sums(kt, slice(512, 1024), 2, sq_eng="act",
                                f_range=range(4))
        stats_rows(s0k0, s1k0, mrow_k[0:1, 0:512], irow_k[0:1, 0:512])
        bmk0, bik0 = bcast(mrow_k[0:1, 0:512], irow_k[0:1, 0:512], "k0")
        pren_k(0, bmk0, bik0)
        KP_EARLY = True
        stats_sums(kt, slice(512, 1024), 2, sq_eng="act", f_range=range(4, 8),
                   psum=(s0k1, s1k1))
        stats_rows(s0k1, s1k1, mrow_k[0:1, 512:1024], irow_k[0:1, 512:1024])
        bmk1, bik1 = bcast(mrow_k[0:1, 512:1024], irow_k[0:1, 512:1024], "k1")
        s0q, s1q = stats_sums(qt, slice(0, 512), 0, use_ss=True)
        stats_rows(s0q, s1q, mrow_q[0:1, :], irow_q[0:1, :])
        bmq, biq = bcast(mrow_q[0:1, :], irow_q[0:1, :], "q")
        pren_q()
        PREN_K1 = (bmk1, bik1)

        # ---- projection group emitters ----
        ph('proj')
        qp = [pqp.tile([128, QS], BF16, tag="qp", name=f"qp{m}") for m in range(8)]
        kp = [pkp.tile([128, NK], BF16, tag="kp", name=f"kp{m}") for m in range(8)]
        vp8 = [pvp.tile([128, 2080], PA, tag="vp", name=f"vp8_{j}") for j in range(4)]
        for j in range(4):
            nc.vector.memset(
                vp8[j][:].rearrange("p (s e) -> p s e", e=65)[:, :, 64:65], 1.0)

        def qp_group(m):
            ps = pp.tile([128, 512], F32, tag="pp")
            if dr_proj:
                for j in range(4):
                    nc.tensor.matmul(ps[:], w3(wq[j])[:, :, m * 128:(m + 1) * 128],
                                     w3(qn8[j]), start=(j == 0), stop=(j == 3),
                                     perf_mode=DR)
            else:
                for j in range(4):
                    for s in range(2):
                        nc.tensor.matmul(
                            ps[:], w3(wq[j])[:, s, m * 128:(m + 1) * 128],
                            w3(qn8[j])[:, s, :],
                            start=(j == 0 and s == 0), stop=(j == 3 and s == 1))
            nc.vector.tensor_copy(qp[m][:], ps[:])

        def kp_group(m, c):
            cs = slice(c * 512, (c + 1) * 512)
            ps = pp.tile([128, 512], F32, tag="pp")
            if dr_proj:
                for j in range(4):
                    nc.tensor.matmul(ps[:], w3(wk[j])[:, :, m * 128:(m + 1) * 128],
                                     w3(kn8[j])[:, :, cs], start=(j == 0),
                                     stop=(j == 3), perf_mode=DR)
            else:
                for j in range(4):
                    for s in range(2):
                        nc.tensor.matmul(
                            ps[:], w3(wk[j])[:, s, m * 128:(m + 1) * 128],
                            w3(kn8[j])[:, s, cs],
                            start=(j == 0 and s == 0), stop=(j == 3 and s == 1))
            nc.vector.tensor_copy(kp[m][:, cs], ps[:])

        def vp_group(t, c):
            ps = pp.tile([128, 512], F32, tag="pp")
            if dr_proj:
                for j in range(4):
                    nc.tensor.matmul(ps[:],
                                     w3(kn8[j])[:, :, t * 128:(t + 1) * 128],
                                     w3(wv[j])[:, :, c * 512:(c + 1) * 512],
                                     start=(j == 0), stop=(j == 3), perf_mode=DR)
            else:
                for j in range(4):
                    for s in range(2):
                        nc.tensor.matmul(
                            ps[:], w3(kn8[j])[:, s, t * 128:(t + 1) * 128],
                            w3(wv[j])[:, s, c * 512:(c + 1) * 512],
                            start=(j == 0 and s == 0), stop=(j == 3 and s == 1))
            base = (t % 2) * 1040 + c * 520
            dst = vp8[t // 2][:, base:base + 520].rearrange(
                "p (s e) -> p s e", e=65)[:, :, 0:64]
            nc.vector.tensor_copy(dst, ps[:].rearrange("p (s e) -> p s e", e=64))

        # prologue: kp-c0 asap, qp next, then pren-k1 and kp-c1
        kp_group(0, 0)
        kp_group(1, 0)
        qp_group(0)
        qp_group(1)
        pren_k(1, *PREN_K1)
        kp_group(0, 1)
        kp_group(1, 1)
        if DEBUG_DUMP:
            nc.sync.dma_start(dbg_qp[:], qp[0][:])
            nc.sync.dma_start(dbg_qn[:], qn8[0][:])

        # per-head-slot extra projection work (spread over the attention ramp)
        extras = {h: [] for h in range(H)}
        for i, m in enumerate(range(2, 8)):
            extras[i].append(("qp", m))
            extras[i].append(("kp", m, 0))
            extras[i].append(("kp", m, 1))
        vi = 0
        for t in range(8):
            for c in range(2):
                extras[vi % 4].append(("vp", t, c))
                vi += 1

        def emit_extras(h):
            for e in extras.get(h, []):
                if e[0] == "qp":
                    qp_group(e[1])
                elif e[0] == "kp":
                    kp_group(e[1], e[2])
                else:
                    vp_group(e[1], e[2])

        # ---- late DMAs ----
        ph('lateDMA')
        qtok = [pqk.tile([128, DV], F32, tag="qtok", name=f"qtok{i}")
                for i in range(4)]
        for i in range(4):
            nc.sync.dma_start(qtok[i][:], qtok_d[i * 128:(i + 1) * 128, :])
        wo = [pwo.tile([128, DV], BF16, tag="wo", name=f"wo{f}") for f in range(8)]
        for f in range(8):
            nc.sync.dma_start(wo[f][:], wo_d[f * 128:(f + 1) * 128, :])
        idn = pool.tile([128, 128], BF16, tag="idn")
        nc.sync.dma_start(idn[:], idn_d[:])

        otok = [pot.tile([128, QS], F32, tag="ot", name=f"otok{i}") for i in range(8)]
        on_fm = [pqp.tile([128, QS], BF16, tag="qp", name=f"onfm{f}")
                 for f in range(8)]

        tcols = {}

        sink = pool.tile([128, 512], BF16, tag="sink")

        def tail_a(qc):
            cols = [pcol.tile([128, 1], F32, tag="col", name=f"c{qc}_{i}")
                    for i in range(6)]
            tcols[qc] = cols
            s0a, s0b, s1a, s1b, mcol, icol = cols
            a, b = otok[2 * qc][:], otok[2 * qc + 1][:]
            # even half on Act (copy/square with accum), odd half on DVE/Pool
            nc.scalar.activation(sink[:], a, AF.Copy, accum_out=s0a[:])
            nc.scalar.activation(sink[:], a, AF.Square, accum_out=s1a[:])
            nc.vector.tensor_reduce(s0b[:], b, mybir.AxisListType.X, OP.add)
            sqb = psq.tile([128, 512], F32R, tag="sq", name=f"osqb{qc}")
            nc.gpsimd.tensor_mul(sqb[:], b, b)
            nc.vector.tensor_reduce(s1b[:], sqb[:].bitcast(F32),
                                    mybir.AxisListType.X, OP.add)

        def tail_b(qc):
            s0a, s0b, s1a, s1b, mcol, icol = tcols[qc]
            nc.vector.tensor_tensor(s0a[:], s0a[:], s0b[:], op=OP.add)
            nc.vector.tensor_tensor(s1a[:], s1a[:], s1b[:], op=OP.add)
            nc.vector.tensor_scalar(mcol[:], s0a[:], 1.0 / DV, None, op0=OP.mult)
            nc.vector.tensor_mul(s0b[:], mcol[:], mcol[:])
            nc.vector.scalar_tensor_tensor(s1b[:], s1a[:], 1.0 / DV, s0b[:],
                                           op0=OP.mult, op1=OP.subtract)
            nc.scalar.activation(s1a[:], s1b[:], AF.Sqrt, bias=epsc[:])
            nc.vector.reciprocal(icol[:], s1a[:])
            negmi = s0a
            nc.vector.tensor_scalar(negmi[:], mcol[:], icol[:], -1.0,
                                    op0=OP.mult, op1=OP.mult)

        ons = {}

        def tail_c(qc):
            s0a, s0b, s1a, s1b, mcol, icol = tcols[qc]
            negmi = s0a
            on = pon.tile([128, NK], BF16, tag="on", name=f"on{qc}")
            ons[qc] = on
            nc.vector.tensor_scalar(on[:, 0:512],
                                    otok[2 * qc][:], icol[:], negmi[:],
                                    op0=OP.mult, op1=OP.add)
            nc.scalar.activation(on[:, 512:1024], otok[2 * qc + 1][:],
                                 AF.Identity, bias=negmi[:], scale=icol[:])
            if DEBUG_DUMP and qc == 0:
                nc.sync.dma_start(dbg_on[:], on[:])

        def tail_d(qc):
            on = ons[qc]
            for f in range(8):
                tp = pss.tile([128, 1024], F32, tag="ss")
                tpv = tp[:, 0:64].bitcast(BF16)
                nc.tensor.transpose(tpv, on[:, f * 128:(f + 1) * 128], idn[:])
                if f % 2 == 0:
                    nc.vector.tensor_copy(
                        on_fm[f][:, qc * 128:(qc + 1) * 128], tpv)
                else:
                    nc.scalar.activation(
                        on_fm[f][:, qc * 128:(qc + 1) * 128], tpv, AF.Copy)

        def tail_e(qc):
            for half in range(2):
                hs = slice(half * 512, (half + 1) * 512)
                ps = pp.tile([128, 512], F32, tag="pp")
                for f in range(8):
                    nc.tensor.matmul(
                        ps[:], on_fm[f][:, qc * 128:(qc + 1) * 128], wo[f][:, hs],
                        start=(f == 0), stop=(f == 7))
                res = pout.tile([128, 512], F32, tag="res")
                nc.vector.scalar_tensor_tensor(res[:], ps[:], 0.0,
                                               otok[2 * qc + half][:],
                                               op0=OP.max, op1=OP.add)
                nc.sync.dma_start(out_d[qc * 128:(qc + 1) * 128, hs], res[:])

        # ---- attention ----
        ph('attn')

        C1 = 0.7071067811865476 / 32.0
        C2 = 0.7071067811865476

        def scores_exp(h, offload=True):
            dt_, po = h // 2, (h % 2) * 64
            at = []
            for j in range(4):
                a = pat.tile([128, 1024], PA, tag="at", name=f"at{h}_{j}")
                ss = pss.tile([128, 1024], F32, tag="ss")
                for half in range(2):
                    k8 = 2 * j + half
                    nc.tensor.matmul(
                        ss[:, half * 512:(half + 1) * 512],
                        kp[dt_][po:po + 64, k8 * 128:(k8 + 1) * 128],
                        qp[dt_][po:po + 64, :], start=True, stop=True)
                if j == 0 and offload and fp8_ctx:
                    # exp(x) ~= (x*c + c)^2 + 1/2 on DVE+Pool; the missing 1/2
                    # is added back in ctx via the half*vpsum rank-1 term
                    xs = pxs.tile([128, 1024], F32, tag="xs", name=f"xs{h}")
                    nc.vector.tensor_scalar(xs[:], ss[:], C1, C2,
                                            op0=OP.mult, op1=OP.add)
                    nc.gpsimd.tensor_mul(a[:], xs[:], xs[:])
                else:
                    nc.scalar.activation(a[:], ss[:], AF.Exp, scale=SCALE)
                at.append(a)
            return at

        def ctx_one(h, qc, at, offload=True):
            sbase = (h // 8) * 520 + (h % 8) * 65
            corr = offload and fp8_ctx
            cc = pcc.tile([128, 65], F32, tag="cc")
            for j in range(4):
                lhs3 = at[j][:].rearrange("p (s n) -> p s n", s=2)
                rhs3 = vp8[j][:].rearrange("p (s n) -> p s n", s=2)
                if fp8_ctx:
                    nc.tensor.matmul(
                        cc[:], lhs3[:, :, qc * 128:(qc + 1) * 128],
                        rhs3[:, :, sbase:sbase + 65],
                        start=(j == 0), stop=(j == 3 and not corr), perf_mode=DR)
                else:
                    for s in range(2):
                        nc.tensor.matmul(
                            cc[:], lhs3[:, s, qc * 128:(qc + 1) * 128],
                            rhs3[:, s, sbase:sbase + 65],
                            start=(j == 0 and s == 0), stop=(j == 3 and s == 1))
            if corr:
                nc.tensor.matmul(cc[:], ones16x[0:1, :],
                                 vpsr[0:1, sbase:sbase + 65],
                                 start=False, stop=True)
            rc = pcol.tile([128, 1], F32, tag="col", name=f"rc{h}_{qc}")
            nc.vector.reciprocal(rc[:], cc[:, 64:65])
            idx, col0 = 2 * qc + h // 8, (h % 8) * 64
            nc.vector.scalar_tensor_tensor(
                otok[idx][:, col0:col0 + 64], cc[:, 0:64], rc[:],
                qtok[qc][:, h * 64:h * 64 + 64],
                op0=OP.mult, op1=OP.add)

        LAG = 4
        vpsr = pool.tile([1, 1040], BF16, tag="vpsr") if OFFLOAD else None
        at_tiles = {}
        for h in range(H):
            at_tiles[h] = scores_exp(h, offload=OFFLOAD)
            emit_extras(h)
            if h == 3 and fp8_ctx and OFFLOAD:
                v3d = vp8[0][:].rearrange("p (s n) -> p s n", s=2)
                h3d = halfcol8[:].rearrange("p (s n) -> p s n", s=2)
                for i in range(4):
                    pv = pp.tile([128, 512], F32, tag="pp")
                    nc.tensor.matmul(pv[0:1, 0:260], h3d,
                                     v3d[:, :, i * 260:(i + 1) * 260],
                                     start=True, stop=True, perf_mode=DR)
                    nc.vector.tensor_copy(vpsr[0:1, i * 260:(i + 1) * 260],
                                          pv[0:1, 0:260])
            if h >= LAG:
                for qc in range(4):
                    ctx_one(h - LAG, qc, at_tiles[h - LAG], offload=OFFLOAD)
        if DEBUG_DUMP:
            nc.sync.dma_start(dbg_kp[:], kp[0][:])
            nc.sync.dma_start(dbg_vp[:], vp8[0][:])
        for h in range(H - LAG, H):
            for qc in range(4):
                ctx_one(h, qc, at_tiles[h], offload=OFFLOAD)
                if h == H - 1:
                    tail_a(qc)
        for qc in range(4):
            tail_b(qc)
        for qc in range(4):
            tail_c(qc)
        for qc in range(4):
            tail_d(qc)
        for qc in range(4):
            tail_e(qc)

    nc.compile()
    return nc


def kernel(**inputs):
    Q = np.asarray(inputs["Q"], np.float32)
    K = np.asarray(inputs["K"], np.float32)
    wq, bq = np.asarray(inputs["wq"], np.float32), np.asarray(inputs["bq"], np.float32)
    wk, bk = np.asarray(inputs["wk"], np.float32), np.asarray(inputs["bk"], np.float32)
    wv, bv = np.asarray(inputs["wv"], np.float32), np.asarray(inputs["bv"], np.float32)
    wo, bo = np.asarray(inputs["wo"], np.float32), np.asarray(inputs["bo"], np.float32)
    gq, betaq = np.asarray(inputs["gq"], np.float32), np.asarray(inputs["betaq"], np.float32)
    gk, betak = np.asarray(inputs["gk"], np.float32), np.asarray(inputs["betak"], np.float32)
    g0, beta0 = np.asarray(inputs["g0"], np.float32), np.asarray(inputs["beta0"], np.float32)

    vq = (betaq @ wq + bq)
    vk = (betak @ wk + bk)
    vv = (betak @ wv + bv)
    vo = (beta0 @ wo + bo)
    zero_bias = (max(np.abs(vq).max(), np.abs(vk).max(), np.abs(vv).max(),
                     np.abs(vo).max()) == 0.0)
    assert zero_bias, "kernel_v3 supports zero-bias reference only"

    key = ("nc", zero_bias, DR_PROJ, FP8_CTX, DEBUG_DUMP)
    if key not in _CACHE:
        _CACHE[key] = _build(zero_bias)
    nc = _CACHE[key]
    _CACHE["nc"] = nc

    BF = ml_dtypes.bfloat16
    F8 = ml_dtypes.float8_e4m3fn

    def packw(w, g):
        ws = (g[:, None] * w).astype(F8 if DR_PROJ else BF)
        return np.ascontiguousarray(
            ws.reshape(4, 2, 128, 1024).transpose(0, 2, 1, 3).reshape(512, 2048))

    shared = {
        "wq": packw(wq, gq), "wk": packw(wk, gk), "wv": packw(wv, gk),
        "wo": np.ascontiguousarray((g0[:, None] * wo).astype(BF)),
        "idn": np.eye(128, dtype=BF),
    }
    in_maps = []
    for c in range(8):
        b, q0 = c // 2, (c % 2) * QS
        m = dict(shared)
        m["qt"] = np.ascontiguousarray(Q[b, q0:q0 + QS, :].T.astype(BF))
        m["qtok"] = np.ascontiguousarray(Q[b, q0:q0 + QS, :])
        m["kt"] = np.ascontiguousarray(K[b].T.astype(BF))
        in_maps.append(m)

    _CACHE["in_map0"] = in_maps[0]
    trace = _CACHE.get("trace", False)
    res = run_bass_kernel_spmd(nc, in_maps, list(range(8)), trace=trace)
    _CACHE["last"] = res

    out = np.empty((B, NQ, DV), np.float32)
    for c in range(8):
        b, q0 = c // 2, (c % 2) * QS
        out[b, q0:q0 + QS, :] = res.results[c]["out"]
    return out


# revision 3
# speedup vs baseline: 1.0007x; 1.0007x over previous
import sys
sys.path.insert(0, "/opt/trn_rl_repo")
import numpy as np
import ml_dtypes
from contextlib import ExitStack

import concourse.bass as bass
import concourse.bacc as bacc
import concourse.tile as tile
from concourse import mybir
from concourse.bass_utils import run_bass_kernel_spmd

F32 = mybir.dt.float32
F32R = mybir.dt.float32r
BF16 = mybir.dt.bfloat16
FP8 = mybir.dt.float8e4
AF = mybir.ActivationFunctionType
OP = mybir.AluOpType
DR = mybir.MatmulPerfMode.DoubleRow

B, NQ, NK, DV, H, D = 4, 1024, 1024, 1024, 16, 64
QS = 512
EPS = 1e-5
SCALE = 1.0 / 32.0
OUT_SHAPE = (QS, DV)

_CACHE = {}
PHASES = []
DR_PROJ = True   # fp8 DoubleRow for Q/K/V projections
FP8_CTX = True   # fp8 at/vp + DR ctx
OFFLOAD = False  # exp offload to DVE/Pool via quadratic
DEBUG_DUMP = False


def expected_shard(full, c):
    b, q0 = c // 2, (c % 2) * QS
    return np.asarray(full)[b, q0:q0 + QS, :]


def _build(zero_bias, dr_proj=None, fp8_ctx=None):
    nc = bacc.Bacc("TRN2", target_bir_lowering=False)
    dr_proj = DR_PROJ if dr_proj is None else dr_proj
    fp8_ctx = FP8_CTX if fp8_ctx is None else fp8_ctx
    PN = FP8 if dr_proj else BF16
    PA = FP8 if fp8_ctx else BF16
    PHASES.clear()

    def ph(label):
        PHASES.append((label, nc.next_id()))

    qt_d = nc.dram_tensor("qt", [DV, QS], BF16, kind="ExternalInput")
    kt_d = nc.dram_tensor("kt", [DV, NK], BF16, kind="ExternalInput")
    qtok_d = nc.dram_tensor("qtok", [QS, DV], F32, kind="ExternalInput")
    # fp8 weights packed for DoubleRow: [512, 2048], row block j holds
    # [128, 2, 1024] = (partition, subtile-pair slot, dv)
    wq_d = nc.dram_tensor("wq", [512, 2048], PN, kind="ExternalInput")
    wk_d = nc.dram_tensor("wk", [512, 2048], PN, kind="ExternalInput")
    wv_d = nc.dram_tensor("wv", [512, 2048], PN, kind="ExternalInput")
    wo_d = nc.dram_tensor("wo", [DV, DV], BF16, kind="ExternalInput")
    idn_d = nc.dram_tensor("idn", [128, 128], BF16, kind="ExternalInput")
    out_d = nc.dram_tensor("out", [QS, DV], F32, kind="ExternalOutput")
    if DEBUG_DUMP:
        dbg_qp = nc.dram_tensor("dbg_qp", [128, QS], BF16, kind="ExternalOutput")
        dbg_kp = nc.dram_tensor("dbg_kp", [128, NK], BF16, kind="ExternalOutput")
        dbg_at = nc.dram_tensor("dbg_at", [128, 1024], PA, kind="ExternalOutput")
        dbg_vp = nc.dram_tensor("dbg_vp", [128, 2080], PA, kind="ExternalOutput")
        dbg_ot = nc.dram_tensor("dbg_ot", [128, QS], F32, kind="ExternalOutput")
        dbg_on = nc.dram_tensor("dbg_on", [128, NK], BF16, kind="ExternalOutput")
        dbg_qn = nc.dram_tensor("dbg_qn", [128, 1024], PN, kind="ExternalOutput")

    with tile.TileContext(nc) as tc, ExitStack() as ctx:
        ctx.enter_context(nc.allow_low_precision(reason="bf16/fp8 by design"))
        P = ctx.enter_context
        pool = P(tc.tile_pool(name="main", bufs=1))
        pw = P(tc.tile_pool(name="w", bufs=12))        # q/k/v weights [128,2048]
        pqt = P(tc.tile_pool(name="pqt", bufs=8))      # qt bf16 [128,512]
        pot = P(tc.tile_pool(name="pot", bufs=8))      # otok f32 [128,512]
        pbig = P(tc.tile_pool(name="big", bufs=8))     # kt bf16 [128,1024]
        pqk = P(tc.tile_pool(name="qtok", bufs=4))     # qtok f32 [128,1024]
        pqn = P(tc.tile_pool(name="qn", bufs=4))       # qn8 [128,1024] fp8
        pkn = P(tc.tile_pool(name="kn", bufs=4))       # kn8 [128,2048] fp8
        pon = P(tc.tile_pool(name="on", bufs=4))       # on bf16 [128,1024]
        pqp = P(tc.tile_pool(name="qp", bufs=8))       # qp bf16 [128,512]
        pkp = P(tc.tile_pool(name="kp", bufs=8))       # kp bf16 [128,1024]
        pvp = P(tc.tile_pool(name="vp", bufs=4))       # vp8 [128,2080] fp8
        pat = P(tc.tile_pool(name="at", bufs=24 if fp8_ctx else 4))  # at tiles
        pxs = P(tc.tile_pool(name="xs", bufs=1))       # exp-offload scratch f32
        pbc = P(tc.tile_pool(name="bc", bufs=4 if (dr_proj and fp8_ctx) else 2))
        psq = P(tc.tile_pool(name="sq", bufs=3 if (dr_proj and fp8_ctx) else 2))
        pcol = P(tc.tile_pool(name="col", bufs=28))    # [128,1] f32
        pcol2 = P(tc.tile_pool(name="col2", bufs=24))  # tail LN-O columns
        pout = P(tc.tile_pool(name="pout", bufs=2))    # out staging f32 [128,512]
        pwo = P(tc.tile_pool(name="pwo", bufs=8))      # wo bf16 [128,1024]
        pp = P(tc.tile_pool(name="pp", bufs=2, space="PSUM"))
        pss = P(tc.tile_pool(name="pss", bufs=2, space="PSUM"))
        pcc = P(tc.tile_pool(name="pcc", bufs=2, space="PSUM"))

        # ---- tiny constants (no DMA where avoidable) ----
        if not zero_bias:
            ones16 = pool.tile([1, 128], BF16, tag="ones16")
            nc.vector.memset(ones16[:], 1.0)
        if OFFLOAD:
            ones16x = pool.tile([1, 128], BF16, tag="ones16x")
            nc.vector.memset(ones16x[:], 1.0)
            halfcol8 = pool.tile([128, 2], FP8, tag="halfcol8")
            nc.vector.memset(halfcol8[:], 0.5)
        onescol = pool.tile([128, 1], BF16, tag="onescol")
        nc.vector.memset(onescol[:], 1.0 / DV)
        onescolr = pool.tile([128, 1], F32R, tag="onescolr")
        nc.vector.memset(onescolr[:].bitcast(F32), 1.0 / DV)
        epst = pool.tile([1, 1], F32, tag="epst")
        nc.vector.memset(epst[:], EPS)
        epsc = pool.tile([128, 1], F32, tag="epsc")
        nc.vector.memset(epsc[:], EPS)
        mrow_q = pool.tile([1, QS], BF16, tag="mrow_q")
        irow_q = pool.tile([1, QS], BF16, tag="irow_q")
        mrow_k = pool.tile([1, NK], BF16, tag="mrow_k")
        irow_k = pool.tile([1, NK], BF16, tag="irow_k")
        m32r = pool.tile([1, 512], F32, tag="m32r")
        varr = pool.tile([1, 512], F32, tag="varr")
        sr = pool.tile([1, 512], F32, tag="sr")

        # ---- input DMAs in consumer order ----
        kt = [pbig.tile([128, NK], BF16, tag="kt", name=f"kt{f}") for f in range(8)]
        for f in range(8):
            nc.sync.dma_start(kt[f][:], kt_d[f * 128:(f + 1) * 128, :])
        qt = [pqt.tile([128, QS], BF16, tag="pq", name=f"qt{f}") for f in range(8)]
        for f in range(8):
            nc.sync.dma_start(qt[f][:], qt_d[f * 128:(f + 1) * 128, :])
        wq = [pw.tile([128, 2048], PN, tag="w", name=f"wq{j}") for j in range(4)]
        for j in range(4):
            nc.sync.dma_start(wq[j][:], wq_d[j * 128:(j + 1) * 128, :])
        wk = [pw.tile([128, 2048], PN, tag="w", name=f"wk{j}") for j in range(4)]
        for j in range(4):
            nc.sync.dma_start(wk[j][:], wk_d[j * 128:(j + 1) * 128, :])
        wv = [pw.tile([128, 2048], PN, tag="w", name=f"wv{j}") for j in range(4)]
        for j in range(4):
            nc.sync.dma_start(wv[j][:], wv_d[j * 128:(j + 1) * 128, :])

        def w3(t):
            return t[:].rearrange("p (s n) -> p s n", s=2)

        def stats_sums(xtiles, cs, n_sq, sq_eng=None, f_range=None, psum=None,
                       use_ss=False):
            if psum is None:
                if use_ss:
                    big = pss.tile([128, 1024], F32, tag="ss")
                    s0t, s1t = big[:, 0:512], big[:, 512:1024]
                else:
                    s0t = pp.tile([128, 512], F32, tag="pp")
                    s1t = pp.tile([128, 512], F32, tag="pp")
                for f in range(8):
                    nc.tensor.matmul(s0t[0:1, :], onescol[:, 0:1], xtiles[f][:, cs],
                                     start=(f == 0), stop=(f == 7))
            else:
                s0t, s1t = psum
            for f in (f_range or range(8)):
                sq = psq.tile([128, 512], F32R, tag="sq", name=f"sq{n_sq}_{f}")
                if sq_eng == "act":
                    nc.scalar.activation(sq[:], xtiles[f][:, cs], AF.Square)
                else:
                    nc.gpsimd.tensor_mul(sq[:], xtiles[f][:, cs], xtiles[f][:, cs])
                nc.tensor.matmul(s1t[0:1, :], onescolr[:, 0:1], sq[:],
                                 start=(f == 0), stop=(f == 7))
            return s0t, s1t

        def stats_rows(s0t, s1t, mrow, irow):
            nc.scalar.activation(mrow[:], s0t[0:1, :], AF.Copy)
            m2 = m32r[0:1, 0:512]
            nc.scalar.activation(m2, s0t[0:1, :], AF.Square)
            var = varr[0:1, 0:512]
            nc.vector.tensor_sub(var, s1t[0:1, :], m2)
            srow = sr[0:1, 0:512]
            nc.scalar.activation(srow, var, AF.Sqrt, bias=epst[:])
            nc.vector.reciprocal(irow[:], srow)

        def bcast(mrow, irow, nm):
            bmt = pbc.tile([128, 512], BF16, tag="bc", name=f"bm{nm}")
            nc.gpsimd.partition_broadcast(bmt[:], mrow[:])
            bit = pbc.tile([128, 512], BF16, tag="bc", name=f"bi{nm}")
            nc.gpsimd.partition_broadcast(bit[:], irow[:])
            return bmt, bit

        # ---- stats + prenorm, finely interleaved for the ramp ----
        ph('statsq')
        kn8 = [pkn.tile([128, 2048], PN, tag="kn", name=f"kn8_{j}") for j in range(4)]
        qn8 = [pqn.tile([128, 1024], PN, tag="qn", name=f"qn8_{j}") for j in range(4)]

        def pren_q():
            for f in range(8):
                j, s = f // 2, f % 2
                dst = qn8[j][:, s * 512:(s + 1) * 512]
                eng = nc.gpsimd if f % 2 == 0 else nc.vector
                eng.tensor_sub(dst, qt[f][:], bmq[:])
                eng.tensor_mul(dst, dst, biq[:])

        def pren_k(c, bmk, bik):
            cs = slice(c * 512, (c + 1) * 512)
            for f in range(8):
                j, s = f // 2, f % 2
                dst = kn8[j][:, s * 1024 + c * 512: s * 1024 + (c + 1) * 512]
                eng = nc.gpsimd if f % 2 == 0 else nc.vector
                eng.tensor_sub(dst, kt[f][:, cs], bmk[:])
                eng.tensor_mul(dst, dst, bik[:])

        ph('statsk')
        s0k0, s1k0 = stats_sums(kt, slice(0, 512), 1)
        s0k1, s1k1 = stats_sums(kt, slice(512, 1024), 2, sq_eng="act",
                                f_range=range(4))
        stats_rows(s0k0, s1k0, mrow_k[0:1, 0:512], irow_k[0:1, 0:512])
        bmk0, bik0 = bcast(mrow_k[0:1, 0:512], irow_k[0:1, 0:512], "k0")
        pren_k(0, bmk0, bik0)
        KP_EARLY = True
        stats_sums(kt, slice(512, 1024), 2, sq_eng="act", f_range=range(4, 8),
                   psum=(s0k1, s1k1))
        stats_rows(s0k1, s1k1, mrow_k[0:1, 512:1024], irow_k[0:1, 512:1024])
        bmk1, bik1 = bcast(mrow_k[0:1, 512:1024], irow_k[0:1, 512:1024], "k1")
        s0q, s1q = stats_sums(qt, slice(0, 512), 0, use_ss=True)
        stats_rows(s0q, s1q, mrow_q[0:1, :], irow_q[0:1, :])
        bmq, biq = bcast(mrow_q[0:1, :], irow_q[0:1, :], "q")
        pren_q()
        PREN_K1 = (bmk1, bik1)

        # ---- projection group emitters ----
        ph('proj')
        qp = [pqp.tile([128, QS], BF16, tag="qp", name=f"qp{m}") for m in range(8)]
        kp = [pkp.tile([128, NK], BF16, tag="kp", name=f"kp{m}") for m in range(8)]
        vp8 = [pvp.tile([128, 2080], PA, tag="vp", name=f"vp8_{j}") for j in range(4)]
        for j in range(4):
            nc.vector.memset(
                vp8[j][:].rearrange("p (s e) -> p s e", e=65)[:, :, 64:65], 1.0)

        def qp_group(m):
            ps = pp.tile([128, 512], F32, tag="pp")
            if dr_proj:
                for j in range(4):
                    nc.tensor.matmul(ps[:], w3(wq[j])[:, :, m * 128:(m + 1) * 128],
                                     w3(qn8[j]), start=(j == 0), stop=(j == 3),
                                     perf_mode=DR)
            else:
                for j in range(4):
                    for s in range(2):
                        nc.tensor.matmul(
                            ps[:], w3(wq[j])[:, s, m * 128:(m + 1) * 128],
                            w3(qn8[j])[:, s, :],
                            start=(j == 0 and s == 0), stop=(j == 3 and s == 1))
            nc.vector.tensor_copy(qp[m][:], ps[:])

        def kp_group(m, c):
            cs = slice(c * 512, (c + 1) * 512)
            ps = pp.tile([128, 512], F32, tag="pp")
            if dr_proj:
                for j in range(4):
                    nc.tensor.matmul(ps[:], w3(wk[j])[:, :, m * 128:(m + 1) * 128],
                                     w3(kn8[j])[:, :, cs], start=(j == 0),
                                     stop=(j == 3), perf_mode=DR)
            else:
                for j in range(4):
                    for s in range(2):
                        nc.tensor.matmul(
                            ps[:], w3(wk[j])[:, s, m * 128:(m + 1) * 128],
                            w3(kn8[j])[:, s, cs],
                            start=(j == 0 and s == 0), stop=(j == 3 and s == 1))
            nc.vector.tensor_copy(kp[m][:, cs], ps[:])

        def vp_group(t, c):
            ps = pp.tile([128, 512], F32, tag="pp")
            if dr_proj:
                for j in range(4):
                    nc.tensor.matmul(ps[:],
                                     w3(kn8[j])[:, :, t * 128:(t + 1) * 128],
                                     w3(wv[j])[:, :, c * 512:(c + 1) * 512],
                                     start=(j == 0), stop=(j == 3), perf_mode=DR)
            else:
                for j in range(4):
                    for s in range(2):
                        nc.tensor.matmul(
                            ps[:], w3(kn8[j])[:, s, t * 128:(t + 1) * 128],
                            w3(wv[j])[:, s, c * 512:(c + 1) * 512],
                            start=(j == 0 and s == 0), stop=(j == 3 and s == 1))
            base = (t % 2) * 1040 + c * 520
            dst = vp8[t // 2][:, base:base + 520].rearrange(
                "p (s e) -> p s e", e=65)[:, :, 0:64]
            nc.vector.tensor_copy(dst, ps[:].rearrange("p (s e) -> p s e", e=64))

        # prologue: kp-c0 asap, qp next, then pren-k1 and kp-c1
        kp_group(0, 0)
        kp_group(1, 0)
        qp_group(0)
        qp_group(1)
        pren_k(1, *PREN_K1)
        kp_group(0, 1)
        kp_group(1, 1)
        if DEBUG_DUMP:
            nc.sync.dma_start(dbg_qp[:], qp[0][:])
            nc.sync.dma_start(dbg_qn[:], qn8[0][:])

        # per-head-slot extra projection work (spread over the attention ramp)
        extras = {h: [] for h in range(H)}
        for i, m in enumerate(range(2, 8)):
            extras[i].append(("qp", m))
            extras[i].append(("kp", m, 0))
            extras[i].append(("kp", m, 1))
        vi = 0
        for t in range(8):
            for c in range(2):
                extras[vi % 4].append(("vp", t, c))
                vi += 1

        def emit_extras(h):
            for e in extras.get(h, []):
                if e[0] == "qp":
                    qp_group(e[1])
                elif e[0] == "kp":
                    kp_group(e[1], e[2])
                else:
                    vp_group(e[1], e[2])

        # ---- late DMAs ----
        ph('lateDMA')
        qtok = [pqk.tile([128, DV], F32, tag="qtok", name=f"qtok{i}")
                for i in range(4)]
        for i in range(4):
            nc.sync.dma_start(qtok[i][:], qtok_d[i * 128:(i + 1) * 128, :])
        wo = [pwo.tile([128, DV], BF16, tag="wo", name=f"wo{f}") for f in range(8)]
        for f in range(8):
            nc.sync.dma_start(wo[f][:], wo_d[f * 128:(f + 1) * 128, :])
        idn = pool.tile([128, 128], BF16, tag="idn")
        nc.sync.dma_start(idn[:], idn_d[:])

        otok = [pot.tile([128, QS], F32, tag="ot", name=f"otok{i}") for i in range(8)]
        on_fm = [pqp.tile([128, QS], BF16, tag="qp", name=f"onfm{f}")
                 for f in range(8)]

        tcols = {}

        sink = pool.tile([128, 512], BF16, tag="sink")

        def tail_a_even(qc):
            cols = [pcol2.tile([128, 1], F32, tag="col2", name=f"c{qc}_{i}")
                    for i in range(6)]
            tcols[qc] = cols
            s0a, s0b, s1a, s1b, mcol, icol = cols
            a = otok[2 * qc][:]
            nc.vector.tensor_reduce(s0a[:], a, mybir.AxisListType.X, OP.add)
            sqa = psq.tile([128, 512], F32R, tag="sq", name=f"osqa{qc}")
            nc.gpsimd.tensor_mul(sqa[:], a, a)
            nc.vector.tensor_reduce(s1a[:], sqa[:].bitcast(F32),
                                    mybir.AxisListType.X, OP.add)

        def tail_a(qc):
            s0a, s0b, s1a, s1b, mcol, icol = tcols[qc]
            b = otok[2 * qc + 1][:]
            # odd half: Act accum + DVE/Pool split
            nc.scalar.activation(sink[:], b, AF.Copy, accum_out=s0b[:])
            sqb = psq.tile([128, 512], F32R, tag="sq", name=f"osqb{qc}")
            nc.gpsimd.tensor_mul(sqb[:], b, b)
            nc.vector.tensor_reduce(s1b[:], sqb[:].bitcast(F32),
                                    mybir.AxisListType.X, OP.add)

        def tail_b(qc):
            s0a, s0b, s1a, s1b, mcol, icol = tcols[qc]
            nc.vector.tensor_tensor(s0a[:], s0a[:], s0b[:], op=OP.add)
            nc.vector.tensor_tensor(s1a[:], s1a[:], s1b[:], op=OP.add)
            nc.vector.tensor_scalar(mcol[:], s0a[:], 1.0 / DV, None, op0=OP.mult)
            nc.vector.tensor_mul(s0b[:], mcol[:], mcol[:])
            nc.vector.scalar_tensor_tensor(s1b[:], s1a[:], 1.0 / DV, s0b[:],
                                           op0=OP.mult, op1=OP.subtract)
            nc.scalar.activation(s1a[:], s1b[:], AF.Sqrt, bias=epsc[:])
            nc.vector.reciprocal(icol[:], s1a[:])
            negmi = s0a
            nc.vector.tensor_scalar(negmi[:], mcol[:], icol[:], -1.0,
                                    op0=OP.mult, op1=OP.mult)

        ons = {}

        def tail_c(qc):
            s0a, s0b, s1a, s1b, mcol, icol = tcols[qc]
            negmi = s0a
            on = pon.tile([128, NK], BF16, tag="on", name=f"on{qc}")
            ons[qc] = on
            nc.vector.tensor_scalar(on[:, 0:512],
                                    otok[2 * qc][:], icol[:], negmi[:],
                                    op0=OP.mult, op1=OP.add)
            nc.scalar.activation(on[:, 512:1024], otok[2 * qc + 1][:],
                                 AF.Identity, bias=negmi[:], scale=icol[:])
            if DEBUG_DUMP and qc == 0:
                nc.sync.dma_start(dbg_on[:], on[:])

        def tail_d(qc):
            on = ons[qc]
            for f in range(8):
                tp = pss.tile([128, 1024], F32, tag="ss")
                tpv = tp[:, 0:64].bitcast(BF16)
                nc.tensor.transpose(tpv, on[:, f * 128:(f + 1) * 128], idn[:])
                if f % 2 == 0:
                    nc.vector.tensor_copy(
                        on_fm[f][:, qc * 128:(qc + 1) * 128], tpv)
                else:
                    nc.scalar.activation(
                        on_fm[f][:, qc * 128:(qc + 1) * 128], tpv, AF.Copy)

        def tail_e(qc):
            for half in range(2):
                hs = slice(half * 512, (half + 1) * 512)
                ps = pp.tile([128, 512], F32, tag="pp")
                for f in range(8):
                    nc.tensor.matmul(
                        ps[:], on_fm[f][:, qc * 128:(qc + 1) * 128], wo[f][:, hs],
                        start=(f == 0), stop=(f == 7))
                res = pout.tile([128, 512], F32, tag="res")
                nc.vector.scalar_tensor_tensor(res[:], ps[:], 0.0,
                                               otok[2 * qc + half][:],
                                               op0=OP.max, op1=OP.add)
                nc.sync.dma_start(out_d[qc * 128:(qc + 1) * 128, hs], res[:])

        # ---- attention ----
        ph('attn')

        C1 = 0.7071067811865476 / 32.0
        C2 = 0.7071067811865476

        def scores_exp(h, offload=True):
            dt_, po = h // 2, (h % 2) * 64
            at = []
            for j in range(4):
                a = pat.tile([128, 1024], PA, tag="at", name=f"at{h}_{j}")
                ss = pss.tile([128, 1024], F32, tag="ss")
                for half in range(2):
                    k8 = 2 * j + half
                    nc.tensor.matmul(
                        ss[:, half * 512:(half + 1) * 512],
                        kp[dt_][po:po + 64, k8 * 128:(k8 + 1) * 128],
                        qp[dt_][po:po + 64, :], start=True, stop=True)
                if j == 0 and offload and fp8_ctx:
                    # exp(x) ~= (x*c + c)^2 + 1/2 on DVE+Pool; the missing 1/2
                    # is added back in ctx via the half*vpsum rank-1 term
                    xs = pxs.tile([128, 1024], F32, tag="xs", name=f"xs{h}")
                    nc.vector.tensor_scalar(xs[:], ss[:], C1, C2,
                                            op0=OP.mult, op1=OP.add)
                    nc.gpsimd.tensor_mul(a[:], xs[:], xs[:])
                else:
                    nc.scalar.activation(a[:], ss[:], AF.Exp, scale=SCALE)
                at.append(a)
            return at

        def ctx_one(h, qc, at, offload=True):
            sbase = (h // 8) * 520 + (h % 8) * 65
            corr = offload and fp8_ctx
            cc = pcc.tile([128, 65], F32, tag="cc")
            for j in range(4):
                lhs3 = at[j][:].rearrange("p (s n) -> p s n", s=2)
                rhs3 = vp8[j][:].rearrange("p (s n) -> p s n", s=2)
                if fp8_ctx:
                    nc.tensor.matmul(
                        cc[:], lhs3[:, :, qc * 128:(qc + 1) * 128],
                        rhs3[:, :, sbase:sbase + 65],
                        start=(j == 0), stop=(j == 3 and not corr), perf_mode=DR)
                else:
                    for s in range(2):
                        nc.tensor.matmul(
                            cc[:], lhs3[:, s, qc * 128:(qc + 1) * 128],
                            rhs3[:, s, sbase:sbase + 65],
                            start=(j == 0 and s == 0), stop=(j == 3 and s == 1))
            if corr:
                nc.tensor.matmul(cc[:], ones16x[0:1, :],
                                 vpsr[0:1, sbase:sbase + 65],
                                 start=False, stop=True)
            rc = pcol.tile([128, 1], F32, tag="col", name=f"rc{h}_{qc}")
            nc.vector.reciprocal(rc[:], cc[:, 64:65])
            idx, col0 = 2 * qc + h // 8, (h % 8) * 64
            nc.vector.scalar_tensor_tensor(
                otok[idx][:, col0:col0 + 64], cc[:, 0:64], rc[:],
                qtok[qc][:, h * 64:h * 64 + 64],
                op0=OP.mult, op1=OP.add)

        LAG = 4
        vpsr = pool.tile([1, 1040], BF16, tag="vpsr") if OFFLOAD else None
        at_tiles = {}
        # ctx catch-up: lag 4 during warm-up, collapse to 1 by h=15
        emit_up_to = {4: 0, 5: 1, 6: 2, 7: 3, 8: 4, 9: 5, 10: 7, 11: 9,
                      12: 11, 13: 12, 14: 13, 15: 14}
        ctx_next = 0
        for h in range(H):
            at_tiles[h] = scores_exp(h, offload=OFFLOAD)
            emit_extras(h)
            if h == 3 and fp8_ctx and OFFLOAD:
                v3d = vp8[0][:].rearrange("p (s n) -> p s n", s=2)
                h3d = halfcol8[:].rearrange("p (s n) -> p s n", s=2)
                for i in range(4):
                    pv = pp.tile([128, 512], F32, tag="pp")
                    nc.tensor.matmul(pv[0:1, 0:260], h3d,
                                     v3d[:, :, i * 260:(i + 1) * 260],
                                     start=True, stop=True, perf_mode=DR)
                    nc.vector.tensor_copy(vpsr[0:1, i * 260:(i + 1) * 260],
                                          pv[0:1, 0:260])
            while ctx_next <= emit_up_to.get(h, -1):
                for qc in range(4):
                    ctx_one(ctx_next, qc, at_tiles[ctx_next], offload=OFFLOAD)
                ctx_next += 1
                if ctx_next == 8:
                    for qc in range(4):
                        tail_a_even(qc)
        if DEBUG_DUMP:
            nc.sync.dma_start(dbg_kp[:], kp[0][:])
            nc.sync.dma_start(dbg_vp[:], vp8[0][:])
        for h in range(ctx_next, H):
            for qc in range(4):
                ctx_one(h, qc, at_tiles[h], offload=OFFLOAD)
                if h == H - 1:
                    tail_a(qc)
        for qc in range(4):
            tail_b(qc)
        for qc in range(4):
            tail_c(qc)
        for qc in range(4):
            tail_d(qc)
        for qc in range(4):
            tail_e(qc)

    nc.compile()
    return nc


def kernel(**inputs):
    Q = np.asarray(inputs["Q"], np.float32)
    K = np.asarray(inputs["K"], np.float32)
    wq, bq = np.asarray(inputs["wq"], np.float32), np.asarray(inputs["bq"], np.float32)
    wk, bk = np.asarray(inputs["wk"], np.float32), np.asarray(inputs["bk"], np.float32)
    wv, bv = np.asarray(inputs["wv"], np.float32), np.asarray(inputs["bv"], np.float32)
    wo, bo = np.asarray(inputs["wo"], np.float32), np.asarray(inputs["bo"], np.float32)
    gq, betaq = np.asarray(inputs["gq"], np.float32), np.asarray(inputs["betaq"], np.float32)
    gk, betak = np.asarray(inputs["gk"], np.float32), np.asarray(inputs["betak"], np.float32)
    g0, beta0 = np.asarray(inputs["g0"], np.float32), np.asarray(inputs["beta0"], np.float32)

    vq = (betaq @ wq + bq)
    vk = (betak @ wk + bk)
    vv = (betak @ wv + bv)
    vo = (beta0 @ wo + bo)
    zero_bias = (max(np.abs(vq).max(), np.abs(vk).max(), np.abs(vv).max(),
                     np.abs(vo).max()) == 0.0)
    assert zero_bias, "kernel_v3 supports zero-bias reference only"

    key = ("nc", zero_bias, DR_PROJ, FP8_CTX, DEBUG_DUMP)
    if key not in _CACHE:
        _CACHE[key] = _build(zero_bias)
    nc = _CACHE[key]
    _CACHE["nc"] = nc

    BF = ml_dtypes.bfloat16
    F8 = ml_dtypes.float8_e4m3fn

    def packw(w, g):
        ws = (g[:, None] * w).astype(F8 if DR_PROJ else BF)
        return np.ascontiguousarray(
            ws.reshape(4, 2, 128, 1024).transpose(0, 2, 1, 3).reshape(512, 2048))

    shared = {
        "wq": packw(wq, gq), "wk": packw(wk, gk), "wv": packw(wv, gk),
        "wo": np.ascontiguousarray((g0[:, None] * wo).astype(BF)),
        "idn": np.eye(128, dtype=BF),
    }
    in_maps = []
    for c in range(8):
        b, q0 = c // 2, (c % 2) * QS
        m = dict(shared)
        m["qt"] = np.ascontiguousarray(Q[b, q0:q0 + QS, :].T.astype(BF))
        m["qtok"] = np.ascontiguousarray(Q[b, q0:q0 + QS, :])
        m["kt"] = np.ascontiguousarray(K[b].T.astype(BF))
        in_maps.append(m)

    _CACHE["in_map0"] = in_maps[0]
    trace = _CACHE.get("trace", False)
    res = run_bass_kernel_spmd(nc, in_maps, list(range(8)), trace=trace)
    _CACHE["last"] = res

    out = np.empty((B, NQ, DV), np.float32)
    for c in range(8):
        b, q0 = c // 2, (c % 2) * QS
        out[b, q0:q0 + QS, :] = res.results[c]["out"]
    return out


# revision 4
# speedup vs baseline: 1.0161x; 1.0154x over previous
import sys
sys.path.insert(0, "/opt/trn_rl_repo")
import numpy as np
import ml_dtypes
from contextlib import ExitStack

import concourse.bass as bass
import concourse.bacc as bacc
import concourse.tile as tile
from concourse import mybir
from concourse.bass_utils import run_bass_kernel_spmd

F32 = mybir.dt.float32
F32R = mybir.dt.float32r
BF16 = mybir.dt.bfloat16
FP8 = mybir.dt.float8e4
AF = mybir.ActivationFunctionType
OP = mybir.AluOpType
DR = mybir.MatmulPerfMode.DoubleRow

B, NQ, NK, DV, H, D = 4, 1024, 1024, 1024, 16, 64
QS = 512
EPS = 1e-5
SCALE = 1.0 / 32.0
OUT_SHAPE = (QS, DV)

_CACHE = {}
PHASES = []
DR_PROJ = True   # fp8 DoubleRow for Q/K/V projections
FP8_CTX = True   # fp8 at/vp + DR ctx
OFFLOAD = False  # exp offload to DVE/Pool via quadratic
DEBUG_DUMP = False


def expected_shard(full, c):
    b, q0 = c // 2, (c % 2) * QS
    return np.asarray(full)[b, q0:q0 + QS, :]


def _build(zero_bias, dr_proj=None, fp8_ctx=None):
    nc = bacc.Bacc("TRN2", target_bir_lowering=False)
    dr_proj = DR_PROJ if dr_proj is None else dr_proj
    fp8_ctx = FP8_CTX if fp8_ctx is None else fp8_ctx
    PN = FP8 if dr_proj else BF16
    PA = FP8 if fp8_ctx else BF16
    PHASES.clear()

    def ph(label):
        PHASES.append((label, nc.next_id()))

    qt_d = nc.dram_tensor("qt", [DV, QS], BF16, kind="ExternalInput")
    kt_d = nc.dram_tensor("kt", [DV, NK], BF16, kind="ExternalInput")
    qtok_d = nc.dram_tensor("qtok", [QS, DV], F32, kind="ExternalInput")
    # fp8 weights packed for DoubleRow: [512, 2048], row block j holds
    # [128, 2, 1024] = (partition, subtile-pair slot, dv)
    wq_d = nc.dram_tensor("wq", [512, 2048], PN, kind="ExternalInput")
    wk_d = nc.dram_tensor("wk", [512, 2048], PN, kind="ExternalInput")
    wv_d = nc.dram_tensor("wv", [512, 2048], PN, kind="ExternalInput")
    wo_d = nc.dram_tensor("wo", [DV, DV], BF16, kind="ExternalInput")
    idn_d = nc.dram_tensor("idn", [128, 128], BF16, kind="ExternalInput")
    out_d = nc.dram_tensor("out", [QS, DV], F32, kind="ExternalOutput")
    if DEBUG_DUMP:
        dbg_qp = nc.dram_tensor("dbg_qp", [128, QS], BF16, kind="ExternalOutput")
        dbg_kp = nc.dram_tensor("dbg_kp", [128, NK], BF16, kind="ExternalOutput")
        dbg_at = nc.dram_tensor("dbg_at", [128, 1024], PA, kind="ExternalOutput")
        dbg_vp = nc.dram_tensor("dbg_vp", [128, 2080], PA, kind="ExternalOutput")
        dbg_ot = nc.dram_tensor("dbg_ot", [128, QS], F32, kind="ExternalOutput")
        dbg_on = nc.dram_tensor("dbg_on", [128, NK], BF16, kind="ExternalOutput")
        dbg_qn = nc.dram_tensor("dbg_qn", [128, 1024], PN, kind="ExternalOutput")

    with tile.TileContext(nc) as tc, ExitStack() as ctx:
        ctx.enter_context(nc.allow_low_precision(reason="bf16/fp8 by design"))
        P = ctx.enter_context
        pool = P(tc.tile_pool(name="main", bufs=1))
        pw = P(tc.tile_pool(name="w", bufs=12))        # q/k/v weights [128,2048]
        pqt = P(tc.tile_pool(name="pqt", bufs=8))      # qt bf16 [128,512]
        pot = P(tc.tile_pool(name="pot", bufs=8))      # otok f32 [128,512]
        pbig = P(tc.tile_pool(name="big", bufs=8))     # kt bf16 [128,1024]
        pqk = P(tc.tile_pool(name="qtok", bufs=4))     # qtok f32 [128,1024]
        pqn = P(tc.tile_pool(name="qn", bufs=4))       # qn8 [128,1024] fp8
        pkn = P(tc.tile_pool(name="kn", bufs=4))       # kn8 [128,2048] fp8
        pon = P(tc.tile_pool(name="on", bufs=4))       # on bf16 [128,1024]
        pqp = P(tc.tile_pool(name="qp", bufs=8))       # qp bf16 [128,512]
        pkp = P(tc.tile_pool(name="kp", bufs=8))       # kp bf16 [128,1024]
        pvp = P(tc.tile_pool(name="vp", bufs=4))       # vp8 [128,2080] fp8
        pat = P(tc.tile_pool(name="at", bufs=24 if fp8_ctx else 4))  # at tiles
        pxs = P(tc.tile_pool(name="xs", bufs=1))       # exp-offload scratch f32
        pbc = P(tc.tile_pool(name="bc", bufs=4 if (dr_proj and fp8_ctx) else 2))
        psq = P(tc.tile_pool(name="sq", bufs=3 if (dr_proj and fp8_ctx) else 2))
        pcol = P(tc.tile_pool(name="col", bufs=28))    # [128,1] f32
        pcol2 = P(tc.tile_pool(name="col2", bufs=24))  # tail LN-O columns
        pout = P(tc.tile_pool(name="pout", bufs=2))    # out staging f32 [128,512]
        pwo = P(tc.tile_pool(name="pwo", bufs=8))      # wo bf16 [128,1024]
        pp = P(tc.tile_pool(name="pp", bufs=2, space="PSUM"))
        pss = P(tc.tile_pool(name="pss", bufs=2, space="PSUM"))
        pcc = P(tc.tile_pool(name="pcc", bufs=2, space="PSUM"))

        # ---- tiny constants (no DMA where avoidable) ----
        if not zero_bias:
            ones16 = pool.tile([1, 128], BF16, tag="ones16")
            nc.vector.memset(ones16[:], 1.0)
        if OFFLOAD:
            ones16x = pool.tile([1, 128], BF16, tag="ones16x")
            nc.vector.memset(ones16x[:], 1.0)
            halfcol8 = pool.tile([128, 2], FP8, tag="halfcol8")
            nc.vector.memset(halfcol8[:], 0.5)
        onescol = pool.tile([128, 1], BF16, tag="onescol")
        nc.vector.memset(onescol[:], 1.0 / DV)
        onescolr = pool.tile([128, 1], F32R, tag="onescolr")
        nc.vector.memset(onescolr[:].bitcast(F32), 1.0 / DV)
        epst = pool.tile([1, 1], F32, tag="epst")
        nc.vector.memset(epst[:], EPS)
        epsc = pool.tile([128, 1], F32, tag="epsc")
        nc.vector.memset(epsc[:], EPS)
        mrow_q = pool.tile([1, QS], BF16, tag="mrow_q")
        irow_q = pool.tile([1, QS], BF16, tag="irow_q")
        mrow_k = pool.tile([1, NK], BF16, tag="mrow_k")
        irow_k = pool.tile([1, NK], BF16, tag="irow_k")
        m32r = pool.tile([1, 512], F32, tag="m32r")
        varr = pool.tile([1, 512], F32, tag="varr")
        sr = pool.tile([1, 512], F32, tag="sr")

        # ---- input DMAs in consumer order ----
        kt = [pbig.tile([128, NK], BF16, tag="kt", name=f"kt{f}") for f in range(8)]
        for f in range(8):
            nc.sync.dma_start(kt[f][:], kt_d[f * 128:(f + 1) * 128, :])
        qt = [pqt.tile([128, QS], BF16, tag="pq", name=f"qt{f}") for f in range(8)]
        for f in range(8):
            nc.sync.dma_start(qt[f][:], qt_d[f * 128:(f + 1) * 128, :])
        wq = [pw.tile([128, 2048], PN, tag="w", name=f"wq{j}") for j in range(4)]
        for j in range(4):
            nc.sync.dma_start(wq[j][:], wq_d[j * 128:(j + 1) * 128, :])
        wk = [pw.tile([128, 2048], PN, tag="w", name=f"wk{j}") for j in range(4)]
        for j in range(4):
            nc.sync.dma_start(wk[j][:], wk_d[j * 128:(j + 1) * 128, :])
        wv = [pw.tile([128, 2048], PN, tag="w", name=f"wv{j}") for j in range(4)]
        for j in range(4):
            nc.sync.dma_start(wv[j][:], wv_d[j * 128:(j + 1) * 128, :])

        def w3(t):
            return t[:].rearrange("p (s n) -> p s n", s=2)

        def stats_sums(xtiles, cs, n_sq, sq_eng=None, f_range=None, psum=None,
                       use_ss=False):
            if psum is None:
                if use_ss:
                    big = pss.tile([128, 1024], F32, tag="ss")
                    s0t, s1t = big[:, 0:512], big[:, 512:1024]
                else:
                    s0t = pp.tile([128, 512], F32, tag="pp")
                    s1t = pp.tile([128, 512], F32, tag="pp")
                for f in range(8):
                    nc.tensor.matmul(s0t[0:1, :], onescol[:, 0:1], xtiles[f][:, cs],
                                     start=(f == 0), stop=(f == 7))
            else:
                s0t, s1t = psum
            for f in (f_range or range(8)):
                sq = psq.tile([128, 512], F32R, tag="sq", name=f"sq{n_sq}_{f}")
                if sq_eng == "act":
                    nc.scalar.activation(sq[:], xtiles[f][:, cs], AF.Square)
                else:
                    nc.gpsimd.tensor_mul(sq[:], xtiles[f][:, cs], xtiles[f][:, cs])
                nc.tensor.matmul(s1t[0:1, :], onescolr[:, 0:1], sq[:],
                                 start=(f == 0), stop=(f == 7))
            return s0t, s1t

        def stats_rows(s0t, s1t, mrow, irow):
            nc.scalar.activation(mrow[:], s0t[0:1, :], AF.Copy)
            m2 = m32r[0:1, 0:512]
            nc.scalar.activation(m2, s0t[0:1, :], AF.Square)
            var = varr[0:1, 0:512]
            nc.vector.tensor_sub(var, s1t[0:1, :], m2)
            srow = sr[0:1, 0:512]
            nc.scalar.activation(srow, var, AF.Sqrt, bias=epst[:])
            nc.vector.reciprocal(irow[:], srow)

        def bcast(mrow, irow, nm):
            bmt = pbc.tile([128, 512], BF16, tag="bc", name=f"bm{nm}")
            nc.gpsimd.partition_broadcast(bmt[:], mrow[:])
            bit = pbc.tile([128, 512], BF16, tag="bc", name=f"bi{nm}")
            nc.gpsimd.partition_broadcast(bit[:], irow[:])
            return bmt, bit

        # ---- stats + prenorm, finely interleaved for the ramp ----
        ph('statsq')
        kn8 = [pkn.tile([128, 2048], PN, tag="kn", name=f"kn8_{j}") for j in range(4)]
        qn8 = [pqn.tile([128, 1024], PN, tag="qn", name=f"qn8_{j}") for j in range(4)]

        def pren_q():
            for f in range(8):
                j, s = f // 2, f % 2
                dst = qn8[j][:, s * 512:(s + 1) * 512]
                eng = nc.gpsimd if f % 2 == 0 else nc.vector
                eng.tensor_sub(dst, qt[f][:], bmq[:])
                eng.tensor_mul(dst, dst, biq[:])

        def pren_k(c, bmk, bik):
            cs = slice(c * 512, (c + 1) * 512)
            for f in range(8):
                j, s = f // 2, f % 2
                dst = kn8[j][:, s * 1024 + c * 512: s * 1024 + (c + 1) * 512]
                eng = nc.gpsimd if f % 2 == 0 else nc.vector
                eng.tensor_sub(dst, kt[f][:, cs], bmk[:])
                eng.tensor_mul(dst, dst, bik[:])

        ph('statsk')
        s0k0, s1k0 = stats_sums(kt, slice(0, 512), 1)
        s0k1, s1k1 = stats_sums(kt, slice(512, 1024), 2, sq_eng="act",
                                f_range=range(4))
        stats_rows(s0k0, s1k0, mrow_k[0:1, 0:512], irow_k[0:1, 0:512])
        bmk0, bik0 = bcast(mrow_k[0:1, 0:512], irow_k[0:1, 0:512], "k0")
        pren_k(0, bmk0, bik0)
        KP_EARLY = True
        stats_sums(kt, slice(512, 1024), 2, sq_eng="act", f_range=range(4, 8),
                   psum=(s0k1, s1k1))
        stats_rows(s0k1, s1k1, mrow_k[0:1, 512:1024], irow_k[0:1, 512:1024])
        bmk1, bik1 = bcast(mrow_k[0:1, 512:1024], irow_k[0:1, 512:1024], "k1")
        s0q, s1q = stats_sums(qt, slice(0, 512), 0, use_ss=True)
        stats_rows(s0q, s1q, mrow_q[0:1, :], irow_q[0:1, :])
        bmq, biq = bcast(mrow_q[0:1, :], irow_q[0:1, :], "q")
        pren_q()
        PREN_K1 = (bmk1, bik1)

        # ---- projection group emitters ----
        ph('proj')
        qp = [pqp.tile([128, QS], BF16, tag="qp", name=f"qp{m}") for m in range(8)]
        kp = [pkp.tile([128, NK], BF16, tag="kp", name=f"kp{m}") for m in range(8)]
        vp8 = [pvp.tile([128, 2080], PA, tag="vp", name=f"vp8_{j}") for j in range(4)]
        for j in range(4):
            nc.vector.memset(
                vp8[j][:].rearrange("p (s e) -> p s e", e=65)[:, :, 64:65], 1.0)

        def qp_group(m, act_copy=False):
            ps = pp.tile([128, 512], F32, tag="pp")
            if dr_proj:
                for j in range(4):
                    nc.tensor.matmul(ps[:], w3(wq[j])[:, :, m * 128:(m + 1) * 128],
                                     w3(qn8[j]), start=(j == 0), stop=(j == 3),
                                     perf_mode=DR)
            else:
                for j in range(4):
                    for s in range(2):
                        nc.tensor.matmul(
                            ps[:], w3(wq[j])[:, s, m * 128:(m + 1) * 128],
                            w3(qn8[j])[:, s, :],
                            start=(j == 0 and s == 0), stop=(j == 3 and s == 1))
            if act_copy:
                nc.scalar.activation(qp[m][:], ps[:], AF.Copy)
            else:
                nc.vector.tensor_copy(qp[m][:], ps[:])

        def kp_group(m, c, act_copy=False):
            cs = slice(c * 512, (c + 1) * 512)
            ps = pp.tile([128, 512], F32, tag="pp")
            if dr_proj:
                for j in range(4):
                    nc.tensor.matmul(ps[:], w3(wk[j])[:, :, m * 128:(m + 1) * 128],
                                     w3(kn8[j])[:, :, cs], start=(j == 0),
                                     stop=(j == 3), perf_mode=DR)
            else:
                for j in range(4):
                    for s in range(2):
                        nc.tensor.matmul(
                            ps[:], w3(wk[j])[:, s, m * 128:(m + 1) * 128],
                            w3(kn8[j])[:, s, cs],
                            start=(j == 0 and s == 0), stop=(j == 3 and s == 1))
            if act_copy:
                nc.scalar.activation(kp[m][:, cs], ps[:], AF.Copy)
            else:
                nc.vector.tensor_copy(kp[m][:, cs], ps[:])

        def vp_group(t, c):
            ps = pp.tile([128, 512], F32, tag="pp")
            if dr_proj:
                for j in range(4):
                    nc.tensor.matmul(ps[:],
                                     w3(kn8[j])[:, :, t * 128:(t + 1) * 128],
                                     w3(wv[j])[:, :, c * 512:(c + 1) * 512],
                                     start=(j == 0), stop=(j == 3), perf_mode=DR)
            else:
                for j in range(4):
                    for s in range(2):
                        nc.tensor.matmul(
                            ps[:], w3(kn8[j])[:, s, t * 128:(t + 1) * 128],
                            w3(wv[j])[:, s, c * 512:(c + 1) * 512],
                            start=(j == 0 and s == 0), stop=(j == 3 and s == 1))
            base = (t % 2) * 1040 + c * 520
            dst = vp8[t // 2][:, base:base + 520].rearrange(
                "p (s e) -> p s e", e=65)[:, :, 0:64]
            nc.vector.tensor_copy(dst, ps[:].rearrange("p (s e) -> p s e", e=64))

        # prologue: kp-c0 asap, qp next, then pren-k1 and kp-c1
        kp_group(0, 0, act_copy=True)
        kp_group(1, 0, act_copy=True)
        qp_group(0, act_copy=True)
        qp_group(1, act_copy=True)
        pren_k(1, *PREN_K1)
        kp_group(0, 1, act_copy=True)
        kp_group(1, 1, act_copy=True)
        if DEBUG_DUMP:
            nc.sync.dma_start(dbg_qp[:], qp[0][:])
            nc.sync.dma_start(dbg_qn[:], qn8[0][:])

        # per-head-slot extra projection work (spread over the attention ramp)
        extras = {h: [] for h in range(H)}
        for i, m in enumerate(range(2, 8)):
            extras[i].append(("qp", m))
            extras[i].append(("kp", m, 0))
            extras[i].append(("kp", m, 1))
        vi = 0
        for t in range(8):
            for c in range(2):
                extras[vi % 4].append(("vp", t, c))
                vi += 1

        def emit_extras(h):
            for e in extras.get(h, []):
                if e[0] == "qp":
                    qp_group(e[1])
                elif e[0] == "kp":
                    kp_group(e[1], e[2])
                else:
                    vp_group(e[1], e[2])

        # ---- late DMAs ----
        ph('lateDMA')
        qtok = [pqk.tile([128, DV], F32, tag="qtok", name=f"qtok{i}")
                for i in range(4)]
        for i in range(4):
            nc.sync.dma_start(qtok[i][:], qtok_d[i * 128:(i + 1) * 128, :])
        wo = [pwo.tile([128, DV], BF16, tag="wo", name=f"wo{f}") for f in range(8)]
        for f in range(8):
            nc.sync.dma_start(wo[f][:], wo_d[f * 128:(f + 1) * 128, :])
        idn = pool.tile([128, 128], BF16, tag="idn")
        nc.sync.dma_start(idn[:], idn_d[:])

        otok = [pot.tile([128, QS], F32, tag="ot", name=f"otok{i}") for i in range(8)]
        on_fm = [pqp.tile([128, QS], BF16, tag="qp", name=f"onfm{f}")
                 for f in range(8)]

        tcols = {}

        sink = pool.tile([128, 512], BF16, tag="sink")

        def tail_a_even(qc):
            cols = [pcol2.tile([128, 1], F32, tag="col2", name=f"c{qc}_{i}")
                    for i in range(6)]
            tcols[qc] = cols
            s0a, s0b, s1a, s1b, mcol, icol = cols
            a = otok[2 * qc][:]
            nc.vector.tensor_reduce(s0a[:], a, mybir.AxisListType.X, OP.add)
            sqa = psq.tile([128, 512], F32R, tag="sq", name=f"osqa{qc}")
            nc.gpsimd.tensor_mul(sqa[:], a, a)
            nc.vector.tensor_reduce(s1a[:], sqa[:].bitcast(F32),
                                    mybir.AxisListType.X, OP.add)

        def tail_a(qc):
            s0a, s0b, s1a, s1b, mcol, icol = tcols[qc]
            b = otok[2 * qc + 1][:]
            # odd half: Act accum + DVE/Pool split
            nc.scalar.activation(sink[:], b, AF.Copy, accum_out=s0b[:])
            sqb = psq.tile([128, 512], F32R, tag="sq", name=f"osqb{qc}")
            nc.gpsimd.tensor_mul(sqb[:], b, b)
            nc.vector.tensor_reduce(s1b[:], sqb[:].bitcast(F32),
                                    mybir.AxisListType.X, OP.add)

        def tail_b(qc):
            s0a, s0b, s1a, s1b, mcol, icol = tcols[qc]
            nc.vector.tensor_tensor(s0a[:], s0a[:], s0b[:], op=OP.add)
            nc.vector.tensor_tensor(s1a[:], s1a[:], s1b[:], op=OP.add)
            nc.vector.tensor_scalar(mcol[:], s0a[:], 1.0 / DV, None, op0=OP.mult)
            nc.vector.tensor_mul(s0b[:], mcol[:], mcol[:])
            nc.vector.scalar_tensor_tensor(s1b[:], s1a[:], 1.0 / DV, s0b[:],
                                           op0=OP.mult, op1=OP.subtract)
            nc.scalar.activation(s1a[:], s1b[:], AF.Sqrt, bias=epsc[:])
            nc.vector.reciprocal(icol[:], s1a[:])
            negmi = s0a
            nc.vector.tensor_scalar(negmi[:], mcol[:], icol[:], -1.0,
                                    op0=OP.mult, op1=OP.mult)

        ons = {}

        def tail_c(qc):
            s0a, s0b, s1a, s1b, mcol, icol = tcols[qc]
            negmi = s0a
            on = pon.tile([128, NK], BF16, tag="on", name=f"on{qc}")
            ons[qc] = on
            nc.vector.tensor_scalar(on[:, 0:512],
                                    otok[2 * qc][:], icol[:], negmi[:],
                                    op0=OP.mult, op1=OP.add)
            nc.scalar.activation(on[:, 512:1024], otok[2 * qc + 1][:],
                                 AF.Identity, bias=negmi[:], scale=icol[:])
            if DEBUG_DUMP and qc == 0:
                nc.sync.dma_start(dbg_on[:], on[:])

        def tail_d(qc):
            on = ons[qc]
            for f in range(8):
                tp = pss.tile([128, 1024], F32, tag="ss")
                tpv = tp[:, 0:64].bitcast(BF16)
                nc.tensor.transpose(tpv, on[:, f * 128:(f + 1) * 128], idn[:])
                if f % 2 == 0:
                    nc.vector.tensor_copy(
                        on_fm[f][:, qc * 128:(qc + 1) * 128], tpv)
                else:
                    nc.scalar.activation(
                        on_fm[f][:, qc * 128:(qc + 1) * 128], tpv, AF.Copy)

        def tail_e(qc):
            for half in range(2):
                hs = slice(half * 512, (half + 1) * 512)
                ps = pp.tile([128, 512], F32, tag="pp")
                for f in range(8):
                    nc.tensor.matmul(
                        ps[:], on_fm[f][:, qc * 128:(qc + 1) * 128], wo[f][:, hs],
                        start=(f == 0), stop=(f == 7))
                res = pout.tile([128, 512], F32, tag="res")
                nc.vector.scalar_tensor_tensor(res[:], ps[:], 0.0,
                                               otok[2 * qc + half][:],
                                               op0=OP.max, op1=OP.add)
                nc.sync.dma_start(out_d[qc * 128:(qc + 1) * 128, hs], res[:])

        # ---- attention ----
        ph('attn')

        C1 = 0.7071067811865476 / 32.0
        C2 = 0.7071067811865476

        def scores_exp(h, offload=True):
            dt_, po = h // 2, (h % 2) * 64
            at = []
            for j in range(4):
                a = pat.tile([128, 1024], PA, tag="at", name=f"at{h}_{j}")
                ss = pss.tile([128, 1024], F32, tag="ss")
                for half in range(2):
                    k8 = 2 * j + half
                    nc.tensor.matmul(
                        ss[:, half * 512:(half + 1) * 512],
                        kp[dt_][po:po + 64, k8 * 128:(k8 + 1) * 128],
                        qp[dt_][po:po + 64, :], start=True, stop=True)
                if j == 0 and offload and fp8_ctx:
                    # exp(x) ~= (x*c + c)^2 + 1/2 on DVE+Pool; the missing 1/2
                    # is added back in ctx via the half*vpsum rank-1 term
                    xs = pxs.tile([128, 1024], F32, tag="xs", name=f"xs{h}")
                    nc.vector.tensor_scalar(xs[:], ss[:], C1, C2,
                                            op0=OP.mult, op1=OP.add)
                    nc.gpsimd.tensor_mul(a[:], xs[:], xs[:])
                else:
                    nc.scalar.activation(a[:], ss[:], AF.Exp, scale=SCALE)
                at.append(a)
            return at

        def ctx_one(h, qc, at, offload=True):
            sbase = (h // 8) * 520 + (h % 8) * 65
            corr = offload and fp8_ctx
            cc = pcc.tile([128, 65], F32, tag="cc")
            for j in range(4):
                lhs3 = at[j][:].rearrange("p (s n) -> p s n", s=2)
                rhs3 = vp8[j][:].rearrange("p (s n) -> p s n", s=2)
                if fp8_ctx:
                    nc.tensor.matmul(
                        cc[:], lhs3[:, :, qc * 128:(qc + 1) * 128],
                        rhs3[:, :, sbase:sbase + 65],
                        start=(j == 0), stop=(j == 3 and not corr), perf_mode=DR)
                else:
                    for s in range(2):
                        nc.tensor.matmul(
                            cc[:], lhs3[:, s, qc * 128:(qc + 1) * 128],
                            rhs3[:, s, sbase:sbase + 65],
                            start=(j == 0 and s == 0), stop=(j == 3 and s == 1))
            if corr:
                nc.tensor.matmul(cc[:], ones16x[0:1, :],
                                 vpsr[0:1, sbase:sbase + 65],
                                 start=False, stop=True)
            rc = pcol.tile([128, 1], F32, tag="col", name=f"rc{h}_{qc}")
            nc.vector.reciprocal(rc[:], cc[:, 64:65])
            idx, col0 = 2 * qc + h // 8, (h % 8) * 64
            nc.vector.scalar_tensor_tensor(
                otok[idx][:, col0:col0 + 64], cc[:, 0:64], rc[:],
                qtok[qc][:, h * 64:h * 64 + 64],
                op0=OP.mult, op1=OP.add)

        LAG = 4
        vpsr = pool.tile([1, 1040], BF16, tag="vpsr") if OFFLOAD else None
        at_tiles = {}
        # ctx catch-up: lag 4 during warm-up, collapse to 1 by h=15
        emit_up_to = {4: 0, 5: 1, 6: 2, 7: 3, 8: 4, 9: 5, 10: 7, 11: 9,
                      12: 11, 13: 12, 14: 13, 15: 14}
        ctx_next = 0
        for h in range(H):
            at_tiles[h] = scores_exp(h, offload=OFFLOAD)
            emit_extras(h)
            if h == 3 and fp8_ctx and OFFLOAD:
                v3d = vp8[0][:].rearrange("p (s n) -> p s n", s=2)
                h3d = halfcol8[:].rearrange("p (s n) -> p s n", s=2)
                for i in range(4):
                    pv = pp.tile([128, 512], F32, tag="pp")
                    nc.tensor.matmul(pv[0:1, 0:260], h3d,
                                     v3d[:, :, i * 260:(i + 1) * 260],
                                     start=True, stop=True, perf_mode=DR)
                    nc.vector.tensor_copy(vpsr[0:1, i * 260:(i + 1) * 260],
                                          pv[0:1, 0:260])
            while ctx_next <= emit_up_to.get(h, -1):
                for qc in range(4):
                    ctx_one(ctx_next, qc, at_tiles[ctx_next], offload=OFFLOAD)
                ctx_next += 1
                if ctx_next == 8:
                    for qc in range(4):
                        tail_a_even(qc)
        if DEBUG_DUMP:
            nc.sync.dma_start(dbg_kp[:], kp[0][:])
            nc.sync.dma_start(dbg_vp[:], vp8[0][:])
        for h in range(ctx_next, H):
            for qc in range(4):
                ctx_one(h, qc, at_tiles[h], offload=OFFLOAD)
                if h == H - 1:
                    tail_a(qc)
        for qc in range(4):
            tail_b(qc)
        for qc in range(4):
            tail_c(qc)
        for qc in range(4):
            tail_d(qc)
        for qc in range(4):
            tail_e(qc)

    nc.compile()
    return nc


def kernel(**inputs):
    Q = np.asarray(inputs["Q"], np.float32)
    K = np.asarray(inputs["K"], np.float32)
    wq, bq = np.asarray(inputs["wq"], np.float32), np.asarray(inputs["bq"], np.float32)
    wk, bk = np.asarray(inputs["wk"], np.float32), np.asarray(inputs["bk"], np.float32)
    wv, bv = np.asarray(inputs["wv"], np.float32), np.asarray(inputs["bv"], np.float32)
    wo, bo = np.asarray(inputs["wo"], np.float32), np.asarray(inputs["bo"], np.float32)
    gq, betaq = np.asarray(inputs["gq"], np.float32), np.asarray(inputs["betaq"], np.float32)
    gk, betak = np.asarray(inputs["gk"], np.float32), np.asarray(inputs["betak"], np.float32)
    g0, beta0 = np.asarray(inputs["g0"], np.float32), np.asarray(inputs["beta0"], np.float32)

    vq = (betaq @ wq + bq)
    vk = (betak @ wk + bk)
    vv = (betak @ wv + bv)
    vo = (beta0 @ wo + bo)
    zero_bias = (max(np.abs(vq).max(), np.abs(vk).max(), np.abs(vv).max(),
                     np.abs(vo).max()) == 0.0)
    assert zero_bias, "kernel_v3 supports zero-bias reference only"

    key = ("nc", zero_bias, DR_PROJ, FP8_CTX, DEBUG_DUMP)
    if key not in _CACHE:
        _CACHE[key] = _build(zero_bias)
    nc = _CACHE[key]
    _CACHE["nc"] = nc

    BF = ml_dtypes.bfloat16
    F8 = ml_dtypes.float8_e4m3fn

    def packw(w, g):
        ws = (g[:, None] * w).astype(F8 if DR_PROJ else BF)
        return np.ascontiguousarray(
            ws.reshape(4, 2, 128, 1024).transpose(0, 2, 1, 3).reshape(512, 2048))

    shared = {
        "wq": packw(wq, gq), "wk": packw(wk, gk), "wv": packw(wv, gk),
        "wo": np.ascontiguousarray((g0[:, None] * wo).astype(BF)),
        "idn": np.eye(128, dtype=BF),
    }
    in_maps = []
    for c in range(8):
        b, q0 = c // 2, (c % 2) * QS
        m = dict(shared)
        m["qt"] = np.ascontiguousarray(Q[b, q0:q0 + QS, :].T.astype(BF))
        m["qtok"] = np.ascontiguousarray(Q[b, q0:q0 + QS, :])
        m["kt"] = np.ascontiguousarray(K[b].T.astype(BF))
        in_maps.append(m)

    _CACHE["in_map0"] = in_maps[0]
    trace = _CACHE.get("trace", False)
    res = run_bass_kernel_spmd(nc, in_maps, list(range(8)), trace=trace)
    _CACHE["last"] = res

    out = np.empty((B, NQ, DV), np.float32)
    for c in range(8):
        b, q0 = c // 2, (c % 2) * QS
        out[b, q0:q0 + QS, :] = res.results[c]["out"]
    return out
